# revision 1
# baseline (speedup 1.0000x reference)
"""Trainium2 Bass kernel for nn_BrickVectorEdgeModel (GNN edge MLP).

Computes, for each batch b and node pair (i, j):
    f   = relu(W_b @ relu(W_a @ bv + b_a + W_xy @ xy + b_xy) + b_b)   per node
    e1  = relu(W1 @ f[j] + W2 @ f[i] + b_ca)                          per edge
    e2  = relu(W_cb @ e1 + b_cb)
    e3  = relu(W_cc @ e2 + b_cc)
    out = W_out @ e3 + b_out                                          (2 channels)

Sharding: the (B=4, N=192) x N edge grid has 768 i-rows; each of the 8
cores takes 96 consecutive rows, which always fall inside a single batch
b = core//2.  Host permutes that batch's 192 nodes so the core's own 96
i-rows come first; every core then runs the identical program (SPMD) on
its own node set.  Matmuls run in float32r (fp32 data, high-half PE pass,
~1e-4 relative error) with on-chip rounding copies as required.
"""

import numpy as np

import concourse.bass as bass
import concourse.mybir as mybir
import concourse.tile as tile
from concourse import bacc
from concourse.bass_utils import run_bass_kernel_spmd

P = 128
H = 512          # hidden width
D = 512          # brick vector dim
B = 4
N = 192          # nodes per batch
NCORES = 8
RLOC = 96        # edge-grid rows per core
EDGES = RLOC * N             # flat edge columns per core (18432)
CHUNK = 512
NCHUNK = EDGES // CHUNK      # 36
NPAD = 256       # node-phase free dim (192 padded to 256 for full-rate f32r)

# weight blob layout (fp16): name -> (offset_cols, size_cols), [128 x WCOLS]
_layout = [
    ("wcat", 5 * H),   # [d_tile(4)+xy_pad(1), 512] stationary tiles for layer a
    ("wb", 4 * H),
    ("w1", 4 * H),
    ("w2", 4 * H),
    ("wcb", 4 * H),
    ("wcc", 4 * H),
    ("wout", 4 * P),   # W_out^T padded from [512,2] to [512,128]
]
OFF = {}
_c = 0
for _n, _s in _layout:
    OFF[_n] = (_c, _s)
    _c += _s
WCOLS = _c

# bias blob layout (fp32): [128 x BCOLS]
_blayout = [("b1", 4), ("bb", 4), ("bca", 4), ("bcb", 4), ("bcc", 4), ("bout", 1)]
BOFF = {}
_c = 0
for _n, _s in _blayout:
    BOFF[_n] = (_c, _s)
    _c += _s
BCOLS = _c


def _to_tiles(w):
    """[K, M] (K = 4*128 contraction) -> [128, 4, M] stationary layout."""
    K, M = w.shape
    return w.reshape(K // P, P, M).transpose(1, 0, 2)


def _pack_weights(W_xy, b_xy, W_a, b_a, W_b, b_b, W_ca, b_ca, W_cb, b_cb,
                  W_cc, b_cc, W_out, b_out):
    blob = np.zeros((P, WCOLS), np.float16)

    def put(name, arr3):  # arr3: [128, n_k, M]
        off, sz = OFF[name]
        blob[:, off:off + sz] = arr3.reshape(P, -1).astype(np.float16)

    wcat = np.zeros((P, 5, H), np.float32)
    wcat[:, :4, :] = _to_tiles(W_a.T.astype(np.float32))      # [512d, 512h]
    wcat[0:2, 4, :] = W_xy.T.astype(np.float32)               # [2, 512]
    put("wcat", wcat)
    put("wb", _to_tiles(W_b.T.astype(np.float32)))
    W1, W2 = W_ca[:, :H], W_ca[:, H:]
    put("w1", _to_tiles(W1.T.astype(np.float32)))
    put("w2", _to_tiles(W2.T.astype(np.float32)))
    put("wcb", _to_tiles(W_cb.T.astype(np.float32)))
    put("wcc", _to_tiles(W_cc.T.astype(np.float32)))
    wout = np.zeros((H, P), np.float32)
    wout[:, 0:2] = W_out.T.astype(np.float32)
    put("wout", _to_tiles(wout))

    bblob = np.zeros((P, BCOLS), np.float32)

    def putb(name, vec):  # [512] -> [128, 4]
        off, sz = BOFF[name]
        bblob[:, off:off + sz] = vec.astype(np.float32).reshape(4, P).T

    putb("b1", np.asarray(b_a) + np.asarray(b_xy))
    putb("bb", b_b)
    putb("bca", b_ca)
    putb("bcb", b_cb)
    putb("bcc", b_cc)
    off, _ = BOFF["bout"]
    bblob[0:2, off] = np.asarray(b_out, np.float32)
    return blob, bblob


def _pack_nodes(bv_b, xy_b, perm):
    """Per-core node blob [128, 5, NPAD] fp16: k-tiles 0-3 = bv^T, 4 = xy^T."""
    nb = np.zeros((P, 5, NPAD), np.float16)
    bvT = bv_b[perm].T.astype(np.float32)          # [512, 192]
    nb[:, 0:4, :N] = bvT.reshape(4, P, N).transpose(1, 0, 2).astype(np.float16)
    nb[0:2, 4, :N] = xy_b[perm].T.astype(np.float16)
    return nb


def _build():
    f32 = mybir.dt.float32
    f32r = mybir.dt.float32r
    Relu = mybir.ActivationFunctionType.Relu
    add = mybir.AluOpType.add
    amax = mybir.AluOpType.max

    f16 = mybir.dt.float16
    nc = bacc.Bacc(None, target_bir_lowering=False)
    wblob = nc.declare_dram_parameter("wblob", [P, WCOLS], f16, isOutput=False)
    bblob = nc.declare_dram_parameter("bblob", [P, BCOLS], f32, isOutput=False)
    nodes = nc.declare_dram_parameter("nodes", [P, 5, NPAD], f16, isOutput=False)
    y = nc.declare_dram_parameter("y", [2, EDGES], f32, isOutput=True)

    with tile.TileContext(nc) as tc:
        with tc.tile_pool(name="wf", bufs=1) as wf, \
             tc.tile_pool(name="stp", bufs=8) as stp, \
             tc.tile_pool(name="wr", bufs=1) as wr, \
             tc.tile_pool(name="ep", bufs=2) as ep, \
             tc.tile_pool(name="outp", bufs=3) as outp, \
             tc.tile_pool(name="psA", bufs=4, space="PSUM") as psA, \
             tc.tile_pool(name="psB", bufs=4, space="PSUM") as psB:

            # biases: small resident fp32 tile
            bias_t = wf.tile([P, BCOLS], f32, tag="bias")
            nc.sync.dma_start(bias_t[:], bblob[:])

            def bias(name, m):
                off, _ = BOFF[name]
                return bias_t[:, off + m:off + m + 1]

            # nodes: stage + round
            nd_f = wf.tile([P, 5, NPAD], f16, tag="nodes")
            nc.sync.dma_start(nd_f[:], nodes[:])
            nd_r = wr.tile([P, 5, NPAD], f32r, tag="nd_r")
            for k in range(5):
                nc.vector.tensor_copy(nd_r[:, k, :], nd_f[:, k, :])

            # weights: two big fp16 staged DMAs (node-phase set, edge set) --
            # DMA triggers cost ~650ns each on the Sync sequencer, so batch
            # them -- then per-k-tile f32r rounding casts off the stages.
            CUT1 = OFF["wb"][0]    # stage A: wcat (needed first)
            CUT2 = OFF["wcb"][0]   # stage B: wb+w1+w2 ; stage C: wcb+wcc+wout
            stA = stp.tile([P, CUT1], f16, tag="stA")
            nc.sync.dma_start(stA[:], wblob[:, :CUT1])
            stB = stp.tile([P, CUT2 - CUT1], f16, tag="stB")
            nc.sync.dma_start(stB[:], wblob[:, CUT1:CUT2])
            stC = stp.tile([P, WCOLS - CUT2], f16, tag="stC")
            nc.sync.dma_start(stC[:], wblob[:, CUT2:])

            def load_w(name, nk, m, eng="v"):
                off, sz = OFF[name]
                assert sz == nk * m
                if off < CUT1:
                    st, base = stA, 0
                elif off < CUT2:
                    st, base = stB, CUT1
                else:
                    st, base = stC, CUT2
                t = wr.tile([P, nk, m], f32r, tag=name)
                for k in range(nk):
                    o = off - base + k * m
                    nc.vector.tensor_copy(t[:, k, :], st[:, o:o + m])
                return t

            wcat = load_w("wcat", 5, H, "v")
            wb = load_w("wb", 4, H, "s")
            w1 = load_w("w1", 4, H, "s")
            w2 = load_w("w2", 4, H, "s")
            wcb = load_w("wcb", 4, H, "v")
            wcc = load_w("wcc", 4, H, "s")
            wout = load_w("wout", 4, P, "v")

            # ---- node phase: f1, f2, then u = W1@f2 (all nodes) and
            #      vpb = W2@f2 + b_ca (per-partition bias source for e1) ----
            f1 = wr.tile([P, 4, NPAD], f32r, tag="f1")
            for m in range(4):
                pt = psA.tile([P, CHUNK], f32, tag="psA")
                for k in range(5):
                    nc.tensor.matmul(pt[:, :NPAD], wcat[:, k, m * P:(m + 1) * P],
                                     nd_r[:, k, :], start=(k == 0), stop=(k == 4))
                nc.scalar.activation(f1[:, m, :], pt[:, :NPAD], Relu,
                                     bias=bias("b1", m), scale=1.0)

            f2 = wr.tile([P, 4, NPAD], f32r, tag="f2")
            for m in range(4):
                pt = psB.tile([P, CHUNK], f32, tag="psB")
                for k in range(4):
                    nc.tensor.matmul(pt[:, :NPAD], wb[:, k, m * P:(m + 1) * P],
                                     f1[:, k, :], start=(k == 0), stop=(k == 3))
                nc.scalar.activation(f2[:, m, :], pt[:, :NPAD], Relu,
                                     bias=bias("bb", m), scale=1.0)

            u = wr.tile([P, 4, NPAD], f32, tag="u")
            vpb = wr.tile([P, 4, NPAD], f32, tag="vpb")
            # interleave u/v per k-tile so e1 of chunk 0 can start early
            for mm in range(4):
                pu = psA.tile([P, CHUNK], f32, tag="psA")
                for k in range(4):
                    nc.tensor.matmul(pu[:, :NPAD], w1[:, k, mm * P:(mm + 1) * P],
                                     f2[:, k, :], start=(k == 0), stop=(k == 3))
                nc.scalar.copy(u[:, mm, :], pu[:, :NPAD])
                pv = psB.tile([P, CHUNK], f32, tag="psB")
                for k in range(4):
                    nc.tensor.matmul(pv[:, :NPAD], w2[:, k, mm * P:(mm + 1) * P],
                                     f2[:, k, :], start=(k == 0), stop=(k == 3))
                nc.vector.tensor_scalar_add(vpb[:, mm, :], pv[:, :NPAD],
                                            bias("bca", mm))

            # ---- edge phase: 36 chunks x 512 flat edge columns ----
            for cc in range(NCHUNK):
                f0 = cc * CHUNK
                e1 = ep.tile([P, 4, CHUNK], f32r, tag="e1")
                r_lo = f0 // N
                r_hi = (f0 + CHUNK - 1) // N
                for rl in range(r_lo, r_hi + 1):
                    cs = max(f0, rl * N)
                    ce = min(f0 + CHUNK, (rl + 1) * N)
                    for kt in range(4):
                        nc.scalar.activation(
                            e1[:, kt, cs - f0:ce - f0],
                            u[:, kt, cs - rl * N:ce - rl * N],
                            Relu, bias=vpb[:, kt, rl:rl + 1], scale=1.0)

                e2 = ep.tile([P, 4, CHUNK], f32r, tag="e2")
                for m in range(4):
                    pt = psA.tile([P, CHUNK], f32, tag="psA")
                    for k in range(4):
                        nc.tensor.matmul(pt[:], wcb[:, k, m * P:(m + 1) * P],
                                         e1[:, k, :], start=(k == 0), stop=(k == 3))
                    nc.vector.tensor_scalar(e2[:, m, :], pt[:],
                                            bias("bcb", m), 0.0, add, amax)

                e3 = ep.tile([P, 4, CHUNK], f32r, tag="e3")
                for m in range(4):
                    pt = psB.tile([P, CHUNK], f32, tag="psB")
                    for k in range(4):
                        nc.tensor.matmul(pt[:], wcc[:, k, m * P:(m + 1) * P],
                                         e2[:, k, :], start=(k == 0), stop=(k == 3))
                    nc.vector.tensor_scalar(e3[:, m, :], pt[:],
                                            bias("bcc", m), 0.0, add, amax)

                po = psA.tile([P, CHUNK], f32, tag="psA")
                for k in range(4):
                    nc.tensor.matmul(po[:], wout[:, k, :], e3[:, k, :],
                                     start=(k == 0), stop=(k == 3))
                ob = outp.tile([2, CHUNK], f32, tag="ob")
                nc.vector.tensor_scalar_add(ob[:], po[:2, :], bias("bout", 0)[:2])
                nc.sync.dma_start(y[:, f0:f0 + CHUNK], ob[:])

    nc.compile()
    return nc


_cache = {}


def _get_nc():
    if "nc" not in _cache:
        _cache["nc"] = _build()
    return _cache["nc"]


def kernel(brick_vectors, xy, W_xy, b_xy, W_a, b_a, W_b, b_b,
           W_ca, b_ca, W_cb, b_cb, W_cc, b_cc, W_out, b_out):
    brick_vectors = np.asarray(brick_vectors, np.float32)
    xy = np.asarray(xy, np.float32)
    blob, bblob = _pack_weights(W_xy, b_xy, W_a, b_a, W_b, b_b, W_ca, b_ca,
                                W_cb, b_cb, W_cc, b_cc, W_out, b_out)

    perms = []
    in_maps = []
    for c in range(NCORES):
        b, half = c // 2, c % 2
        perm = np.concatenate([np.arange(96) + 96 * half,
                               np.arange(96) + 96 * (1 - half)])
        perms.append((b, perm))
        in_maps.append({
            "wblob": blob,
            "bblob": bblob,
            "nodes": _pack_nodes(brick_vectors[b], xy[b], perm),
        })

    nc = _get_nc()
    res = run_bass_kernel_spmd(nc, in_maps, list(range(NCORES)))

    out = np.empty((B, N, N, 2), np.float32)
    for c in range(NCORES):
        b, perm = perms[c]
        yc = res.results[c]["y"].reshape(2, RLOC, N)       # [2, rl, jj]
        out[b][np.ix_(perm[:RLOC], perm)] = yc.transpose(1, 2, 0)
    return out


# revision 2
# speedup vs baseline: 1.0053x; 1.0053x over previous
"""Trainium2 Bass kernel for nn_BrickVectorEdgeModel (GNN edge MLP).

Computes, for each batch b and node pair (i, j):
    f   = relu(W_b @ relu(W_a @ bv + b_a + W_xy @ xy + b_xy) + b_b)   per node
    e1  = relu(W1 @ f[j] + W2 @ f[i] + b_ca)                          per edge
    e2  = relu(W_cb @ e1 + b_cb)
    e3  = relu(W_cc @ e2 + b_cc)
    out = W_out @ e3 + b_out                                          (2 channels)

Sharding: the (B=4, N=192) x N edge grid has 768 i-rows; each of the 8
cores takes 96 consecutive rows, which always fall inside a single batch
b = core//2.  Host permutes that batch's 192 nodes so the core's own 96
i-rows come first; every core then runs the identical program (SPMD) on
its own node set.  Matmuls run in float32r (fp32 data, high-half PE pass,
~1e-4 relative error) with on-chip rounding copies as required.
"""

import numpy as np

import concourse.bass as bass
import concourse.mybir as mybir
import concourse.tile as tile
from concourse import bacc
from concourse.bass_utils import run_bass_kernel_spmd

P = 128
H = 512          # hidden width
D = 512          # brick vector dim
B = 4
N = 192          # nodes per batch
NCORES = 8
RLOC = 96        # edge-grid rows per core
EDGES = RLOC * N             # flat edge columns per core (18432)
CHUNK = 512
NCHUNK = EDGES // CHUNK      # 36
NPAD = 256       # node-phase free dim (192 padded to 256 for full-rate f32r)

# weight blob layout (fp16): name -> (offset_cols, size_cols), [128 x WCOLS]
_layout = [
    ("wcat", 5 * H),   # [d_tile(4)+xy_pad(1), 512] stationary tiles for layer a
    ("wb", 4 * H),
    ("w1", 4 * H),
    ("w2", 4 * H),
    ("wcb", 4 * H),
    ("wcc", 4 * H),
    ("wout", 4 * P),   # W_out^T padded from [512,2] to [512,128]
]
OFF = {}
_c = 0
for _n, _s in _layout:
    OFF[_n] = (_c, _s)
    _c += _s
WCOLS = _c

# bias blob layout (fp32): [128 x BCOLS]
_blayout = [("b1", 4), ("bb", 4), ("bca", 4), ("bcb", 4), ("bcc", 4), ("bout", 1)]
BOFF = {}
_c = 0
for _n, _s in _blayout:
    BOFF[_n] = (_c, _s)
    _c += _s
BCOLS = _c


def _to_tiles(w):
    """[K, M] (K = 4*128 contraction) -> [128, 4, M] stationary layout."""
    K, M = w.shape
    return w.reshape(K // P, P, M).transpose(1, 0, 2)


def _pack_weights(W_xy, b_xy, W_a, b_a, W_b, b_b, W_ca, b_ca, W_cb, b_cb,
                  W_cc, b_cc, W_out, b_out):
    blob = np.zeros((P, WCOLS), np.float16)

    def put(name, arr3):  # arr3: [128, n_k, M]
        off, sz = OFF[name]
        blob[:, off:off + sz] = arr3.reshape(P, -1).astype(np.float16)

    wcat = np.zeros((P, 5, H), np.float32)
    wcat[:, :4, :] = _to_tiles(W_a.T.astype(np.float32))      # [512d, 512h]
    wcat[0:2, 4, :] = W_xy.T.astype(np.float32)               # [2, 512]
    put("wcat", wcat)
    put("wb", _to_tiles(W_b.T.astype(np.float32)))
    W1, W2 = W_ca[:, :H], W_ca[:, H:]
    put("w1", _to_tiles(W1.T.astype(np.float32)))
    put("w2", _to_tiles(W2.T.astype(np.float32)))
    put("wcb", _to_tiles(W_cb.T.astype(np.float32)))
    put("wcc", _to_tiles(W_cc.T.astype(np.float32)))
    wout = np.zeros((H, P), np.float32)
    wout[:, 0:2] = W_out.T.astype(np.float32)
    put("wout", _to_tiles(wout))

    bblob = np.zeros((P, BCOLS), np.float32)

    def putb(name, vec):  # [512] -> [128, 4]
        off, sz = BOFF[name]
        bblob[:, off:off + sz] = vec.astype(np.float32).reshape(4, P).T

    putb("b1", np.asarray(b_a) + np.asarray(b_xy))
    putb("bb", b_b)
    putb("bca", b_ca)
    putb("bcb", b_cb)
    putb("bcc", b_cc)
    off, _ = BOFF["bout"]
    bblob[0:2, off] = np.asarray(b_out, np.float32)
    return blob, bblob


def _pack_nodes(bv_b, xy_b, perm):
    """Per-core node blob [128, 5, NPAD] fp16: k-tiles 0-3 = bv^T, 4 = xy^T."""
    nb = np.zeros((P, 5, NPAD), np.float16)
    bvT = bv_b[perm].T.astype(np.float32)          # [512, 192]
    nb[:, 0:4, :N] = bvT.reshape(4, P, N).transpose(1, 0, 2).astype(np.float16)
    nb[0:2, 4, :N] = xy_b[perm].T.astype(np.float16)
    return nb


def _build():
    f32 = mybir.dt.float32
    f32r = mybir.dt.float32r
    Relu = mybir.ActivationFunctionType.Relu
    add = mybir.AluOpType.add
    amax = mybir.AluOpType.max

    f16 = mybir.dt.float16
    nc = bacc.Bacc(None, target_bir_lowering=False)
    wblob = nc.declare_dram_parameter("wblob", [P, WCOLS], f16, isOutput=False)
    bblob = nc.declare_dram_parameter("bblob", [P, BCOLS], f32, isOutput=False)
    nodes = nc.declare_dram_parameter("nodes", [P, 5, NPAD], f16, isOutput=False)
    y = nc.declare_dram_parameter("y", [2, EDGES], f32, isOutput=True)

    with tile.TileContext(nc) as tc:
        with tc.tile_pool(name="wf", bufs=1) as wf, \
             tc.tile_pool(name="stp", bufs=8) as stp, \
             tc.tile_pool(name="wr", bufs=1) as wr, \
             tc.tile_pool(name="ep", bufs=2) as ep, \
             tc.tile_pool(name="outp", bufs=3) as outp, \
             tc.tile_pool(name="psA", bufs=4, space="PSUM") as psA, \
             tc.tile_pool(name="psB", bufs=4, space="PSUM") as psB:

            # biases: small resident fp32 tile (declared below, loaded after
            # the critical-path weight stage A)
            bias_t = wf.tile([P, BCOLS], f32, tag="bias")

            def bias(name, m):
                off, _ = BOFF[name]
                return bias_t[:, off + m:off + m + 1]

            # nodes: stage + round (casts interleaved with wcat below)
            nd_f = wf.tile([P, 5, NPAD], f16, tag="nodes")
            nc.sync.dma_start(nd_f[:], nodes[:])
            nd_r = wr.tile([P, 5, NPAD], f32r, tag="nd_r")

            # weights: two big fp16 staged DMAs (node-phase set, edge set) --
            # DMA triggers cost ~650ns each on the Sync sequencer, so batch
            # them -- then per-k-tile f32r rounding casts off the stages.
            CUT1 = OFF["wb"][0]    # stage A: wcat (needed first)
            CUT2 = OFF["wcb"][0]   # stage B: wb+w1+w2 ; stage C: wcb+wcc+wout
            stA = stp.tile([P, CUT1], f16, tag="stA")
            nc.sync.dma_start(stA[:], wblob[:, :CUT1])
            nc.sync.dma_start(bias_t[:], bblob[:])
            stB = stp.tile([P, CUT2 - CUT1], f16, tag="stB")
            nc.sync.dma_start(stB[:], wblob[:, CUT1:CUT2])
            stC = stp.tile([P, WCOLS - CUT2], f16, tag="stC")
            nc.sync.dma_start(stC[:], wblob[:, CUT2:])

            def load_w(name, nk, m, eng="v", extra=None):
                off, sz = OFF[name]
                assert sz == nk * m
                if off < CUT1:
                    st, base = stA, 0
                elif off < CUT2:
                    st, base = stB, CUT1
                else:
                    st, base = stC, CUT2
                t = wr.tile([P, nk, m], f32r, tag=name)
                for k in range(nk):
                    o = off - base + k * m
                    e = nc.vector if eng == "v" else nc.gpsimd
                    e.tensor_copy(t[:, k, :], st[:, o:o + m])
                    if extra is not None:
                        extra(k)
                return t

            # interleave nd-cast after each wcat-cast: f1's k-th MM needs both
            def _nd_cast(k):
                nc.vector.tensor_copy(nd_r[:, k, :], nd_f[:, k, :])

            wcat = load_w("wcat", 5, H, "v", extra=_nd_cast)
            wb = load_w("wb", 4, H, "v")
            w1 = load_w("w1", 4, H, "v")
            w2 = load_w("w2", 4, H, "v")
            wcb = load_w("wcb", 4, H, "v")
            wcc = load_w("wcc", 4, H, "v")
            wout = load_w("wout", 4, P, "v")

            # ---- node phase: f1, f2, then u = W1@f2 (all nodes) and
            #      vpb = W2@f2 + b_ca (per-partition bias source for e1) ----
            f1 = wr.tile([P, 4, NPAD], f32r, tag="f1")
            for m in range(4):
                pt = psA.tile([P, CHUNK], f32, tag="psA")
                for k in range(5):
                    nc.tensor.matmul(pt[:, :NPAD], wcat[:, k, m * P:(m + 1) * P],
                                     nd_r[:, k, :], start=(k == 0), stop=(k == 4))
                nc.scalar.activation(f1[:, m, :], pt[:, :NPAD], Relu,
                                     bias=bias("b1", m), scale=1.0)

            f2 = wr.tile([P, 4, NPAD], f32r, tag="f2")
            for m in range(4):
                pt = psB.tile([P, CHUNK], f32, tag="psB")
                for k in range(4):
                    nc.tensor.matmul(pt[:, :NPAD], wb[:, k, m * P:(m + 1) * P],
                                     f1[:, k, :], start=(k == 0), stop=(k == 3))
                nc.scalar.activation(f2[:, m, :], pt[:, :NPAD], Relu,
                                     bias=bias("bb", m), scale=1.0)

            u = wr.tile([P, 4, NPAD], f32, tag="u")
            vpb = wr.tile([P, 4, NPAD], f32, tag="vpb")
            # interleave u/v per k-tile so e1 of chunk 0 can start early
            for mm in range(4):
                pu = psA.tile([P, CHUNK], f32, tag="psA")
                for k in range(4):
                    nc.tensor.matmul(pu[:, :NPAD], w1[:, k, mm * P:(mm + 1) * P],
                                     f2[:, k, :], start=(k == 0), stop=(k == 3))
                nc.scalar.copy(u[:, mm, :], pu[:, :NPAD])
                pv = psB.tile([P, CHUNK], f32, tag="psB")
                for k in range(4):
                    nc.tensor.matmul(pv[:, :NPAD], w2[:, k, mm * P:(mm + 1) * P],
                                     f2[:, k, :], start=(k == 0), stop=(k == 3))
                nc.vector.tensor_scalar_add(vpb[:, mm, :], pv[:, :NPAD],
                                            bias("bca", mm))

            # ---- edge phase: 36 chunks x 512 flat edge columns ----
            for cc in range(NCHUNK):
                f0 = cc * CHUNK
                e1 = ep.tile([P, 4, CHUNK], f32r, tag="e1")
                r_lo = f0 // N
                r_hi = (f0 + CHUNK - 1) // N
                for kt in range(4):
                    for rl in range(r_lo, r_hi + 1):
                        cs = max(f0, rl * N)
                        ce = min(f0 + CHUNK, (rl + 1) * N)
                        nc.scalar.activation(
                            e1[:, kt, cs - f0:ce - f0],
                            u[:, kt, cs - rl * N:ce - rl * N],
                            Relu, bias=vpb[:, kt, rl:rl + 1], scale=1.0)

                e2 = ep.tile([P, 4, CHUNK], f32r, tag="e2")
                for m in range(4):
                    pt = psA.tile([P, CHUNK], f32, tag="psA")
                    for k in range(4):
                        nc.tensor.matmul(pt[:], wcb[:, k, m * P:(m + 1) * P],
                                         e1[:, k, :], start=(k == 0), stop=(k == 3))
                    nc.vector.tensor_scalar(e2[:, m, :], pt[:],
                                            bias("bcb", m), 0.0, add, amax)

                e3 = ep.tile([P, 4, CHUNK], f32r, tag="e3")
                for m in range(4):
                    pt = psB.tile([P, CHUNK], f32, tag="psB")
                    for k in range(4):
                        nc.tensor.matmul(pt[:], wcc[:, k, m * P:(m + 1) * P],
                                         e2[:, k, :], start=(k == 0), stop=(k == 3))
                    nc.vector.tensor_scalar(e3[:, m, :], pt[:],
                                            bias("bcc", m), 0.0, add, amax)

                po = psA.tile([P, CHUNK], f32, tag="psA")
                for k in range(4):
                    nc.tensor.matmul(po[:], wout[:, k, :], e3[:, k, :],
                                     start=(k == 0), stop=(k == 3))
                ob = outp.tile([2, CHUNK], f32, tag="ob")
                nc.vector.tensor_scalar_add(ob[:], po[:2, :], bias("bout", 0)[:2])
                nc.sync.dma_start(y[:, f0:f0 + CHUNK], ob[:])

    nc.compile()
    return nc


_cache = {}


def _get_nc():
    if "nc" not in _cache:
        _cache["nc"] = _build()
    return _cache["nc"]


def kernel(brick_vectors, xy, W_xy, b_xy, W_a, b_a, W_b, b_b,
           W_ca, b_ca, W_cb, b_cb, W_cc, b_cc, W_out, b_out):
    brick_vectors = np.asarray(brick_vectors, np.float32)
    xy = np.asarray(xy, np.float32)
    blob, bblob = _pack_weights(W_xy, b_xy, W_a, b_a, W_b, b_b, W_ca, b_ca,
                                W_cb, b_cb, W_cc, b_cc, W_out, b_out)

    perms = []
    in_maps = []
    for c in range(NCORES):
        b, half = c // 2, c % 2
        perm = np.concatenate([np.arange(96) + 96 * half,
                               np.arange(96) + 96 * (1 - half)])
        perms.append((b, perm))
        in_maps.append({
            "wblob": blob,
            "bblob": bblob,
            "nodes": _pack_nodes(brick_vectors[b], xy[b], perm),
        })

    nc = _get_nc()
    res = run_bass_kernel_spmd(nc, in_maps, list(range(NCORES)))

    out = np.empty((B, N, N, 2), np.float32)
    for c in range(NCORES):
        b, perm = perms[c]
        yc = res.results[c]["y"].reshape(2, RLOC, N)       # [2, rl, jj]
        out[b][np.ix_(perm[:RLOC], perm)] = yc.transpose(1, 2, 0)
    return out


# revision 3
# speedup vs baseline: 1.0676x; 1.0619x over previous
"""Trainium2 Bass kernel for nn_BrickVectorEdgeModel (GNN edge MLP).

Computes, for each batch b and node pair (i, j):
    f   = relu(W_b @ relu(W_a @ bv + b_a + W_xy @ xy + b_xy) + b_b)   per node
    e1  = relu(W1 @ f[j] + W2 @ f[i] + b_ca)                          per edge
    e2  = relu(W_cb @ e1 + b_cb)
    e3  = relu(W_cc @ e2 + b_cc)
    out = W_out @ e3 + b_out                                          (2 channels)

Sharding: the (B=4, N=192) x N edge grid has 768 i-rows; each of the 8
cores takes 96 consecutive rows, which always fall inside a single batch
b = core//2.  Host permutes that batch's 192 nodes so the core's own 96
i-rows come first; every core then runs the identical program (SPMD) on
its own node set.  Matmuls run in float32r (fp32 data, high-half PE pass,
~1e-4 relative error) with on-chip rounding copies as required.
"""

import numpy as np

import concourse.bass as bass
import concourse.mybir as mybir
import concourse.tile as tile
from concourse import bacc
from concourse.bass_utils import run_bass_kernel_spmd

P = 128
H = 512          # hidden width
D = 512          # brick vector dim
B = 4
N = 192          # nodes per batch
NCORES = 8
RLOC = 96        # edge-grid rows per core
EDGES = RLOC * N             # flat edge columns per core (18432)
CHUNK = 512
NCHUNK = EDGES // CHUNK      # 36
NPAD = 256       # node-phase free dim (192 padded to 256 for full-rate f32r)

# weight blob layout (fp16): name -> (offset_cols, size_cols), [128 x WCOLS]
_layout = [
    ("wcat", 5 * H),   # [d_tile(4)+xy_pad(1), 512] stationary tiles for layer a
    ("wb", 4 * H),
    ("w1", 4 * H),
    ("w2", 4 * H),
    ("wcb", 4 * H),
    ("wcc", 4 * H),
    ("wout", 4 * P),   # W_out^T padded from [512,2] to [512,128]
]
OFF = {}
_c = 0
for _n, _s in _layout:
    OFF[_n] = (_c, _s)
    _c += _s
WCOLS = _c

# bias blob layout (fp32): [128 x BCOLS]
_blayout = [("b1", 4), ("bb", 4), ("bca", 4), ("bcb", 4), ("bcc", 4), ("bout", 1)]
BOFF = {}
_c = 0
for _n, _s in _blayout:
    BOFF[_n] = (_c, _s)
    _c += _s
BCOLS = _c


def _to_tiles(w):
    """[K, M] (K = 4*128 contraction) -> [128, 4, M] stationary layout."""
    K, M = w.shape
    return w.reshape(K // P, P, M).transpose(1, 0, 2)


def _pack_weights(W_xy, b_xy, W_a, b_a, W_b, b_b, W_ca, b_ca, W_cb, b_cb,
                  W_cc, b_cc, W_out, b_out):
    blob = np.zeros((P, WCOLS), np.float16)

    def put(name, arr3):  # arr3: [128, n_k, M]
        off, sz = OFF[name]
        blob[:, off:off + sz] = arr3.reshape(P, -1).astype(np.float16)

    wcat = np.zeros((P, 5, H), np.float32)
    wcat[:, :4, :] = _to_tiles(W_a.T.astype(np.float32))      # [512d, 512h]
    wcat[0:2, 4, :] = W_xy.T.astype(np.float32)               # [2, 512]
    put("wcat", wcat)
    put("wb", _to_tiles(W_b.T.astype(np.float32)))
    W1, W2 = W_ca[:, :H], W_ca[:, H:]
    put("w1", _to_tiles(W1.T.astype(np.float32)))
    put("w2", _to_tiles(W2.T.astype(np.float32)))
    put("wcb", _to_tiles(W_cb.T.astype(np.float32)))
    put("wcc", _to_tiles(W_cc.T.astype(np.float32)))
    wout = np.zeros((H, P), np.float32)
    wout[:, 0:2] = W_out.T.astype(np.float32)
    put("wout", _to_tiles(wout))

    bblob = np.zeros((P, BCOLS), np.float32)

    def putb(name, vec):  # [512] -> [128, 4]
        off, sz = BOFF[name]
        bblob[:, off:off + sz] = vec.astype(np.float32).reshape(4, P).T

    putb("b1", np.asarray(b_a) + np.asarray(b_xy))
    putb("bb", b_b)
    putb("bca", b_ca)
    putb("bcb", b_cb)
    putb("bcc", b_cc)
    off, _ = BOFF["bout"]
    bblob[0:2, off] = np.asarray(b_out, np.float32)
    return blob, bblob


def _pack_nodes(bv_b, xy_b, perm):
    """Per-core node blob [128, 5, NPAD] fp16: k-tiles 0-3 = bv^T, 4 = xy^T."""
    nb = np.zeros((P, 5, NPAD), np.float16)
    bvT = bv_b[perm].T.astype(np.float32)          # [512, 192]
    nb[:, 0:4, :N] = bvT.reshape(4, P, N).transpose(1, 0, 2).astype(np.float16)
    nb[0:2, 4, :N] = xy_b[perm].T.astype(np.float16)
    return nb


def _build():
    f32 = mybir.dt.float32
    f32r = mybir.dt.float32r
    Relu = mybir.ActivationFunctionType.Relu
    add = mybir.AluOpType.add
    amax = mybir.AluOpType.max

    f16 = mybir.dt.float16
    nc = bacc.Bacc(None, target_bir_lowering=False)
    wblob = nc.declare_dram_parameter("wblob", [P, WCOLS], f16, isOutput=False)
    bblob = nc.declare_dram_parameter("bblob", [P, BCOLS], f32, isOutput=False)
    nodes = nc.declare_dram_parameter("nodes", [P, 5, NPAD], f16, isOutput=False)
    y = nc.declare_dram_parameter("y", [2, EDGES], f32, isOutput=True)

    with tile.TileContext(nc) as tc:
        with tc.tile_pool(name="wf", bufs=1) as wf, \
             tc.tile_pool(name="stp", bufs=8) as stp, \
             tc.tile_pool(name="wr", bufs=1) as wr, \
             tc.tile_pool(name="ep", bufs=2) as ep, \
             tc.tile_pool(name="outp", bufs=3) as outp, \
             tc.tile_pool(name="psA", bufs=4, space="PSUM") as psA, \
             tc.tile_pool(name="psB", bufs=4, space="PSUM") as psB:

            # biases: small resident fp32 tile (declared below, loaded after
            # the critical-path weight stage A)
            bias_t = wf.tile([P, BCOLS], f32, tag="bias")

            def bias(name, m):
                off, _ = BOFF[name]
                return bias_t[:, off + m:off + m + 1]

            # nodes: stage + round (casts interleaved with wcat below)
            nd_f = wf.tile([P, 5, NPAD], f16, tag="nodes")
            nc.sync.dma_start(nd_f[:], nodes[:])
            nd_r = wr.tile([P, 5, NPAD], f32r, tag="nd_r")

            # weights: two big fp16 staged DMAs (node-phase set, edge set) --
            # DMA triggers cost ~650ns each on the Sync sequencer, so batch
            # them -- then per-k-tile f32r rounding casts off the stages.
            CUT1 = OFF["wb"][0]    # stage A: wcat (needed first)
            CUT2 = OFF["wcb"][0]   # stage B: wb+w1+w2 ; stage C: wcb+wcc+wout
            stA = stp.tile([P, CUT1], f16, tag="stA")
            nc.sync.dma_start(stA[:], wblob[:, :CUT1])
            nc.sync.dma_start(bias_t[:], bblob[:])
            stB = stp.tile([P, CUT2 - CUT1], f16, tag="stB")
            nc.sync.dma_start(stB[:], wblob[:, CUT1:CUT2])
            stC = stp.tile([P, WCOLS - CUT2], f16, tag="stC")
            nc.sync.dma_start(stC[:], wblob[:, CUT2:])

            def load_w(name, nk, m, eng="v", extra=None):
                off, sz = OFF[name]
                assert sz == nk * m
                if off < CUT1:
                    st, base = stA, 0
                elif off < CUT2:
                    st, base = stB, CUT1
                else:
                    st, base = stC, CUT2
                t = wr.tile([P, nk, m], f32r, tag=name)
                for k in range(nk):
                    o = off - base + k * m
                    e = nc.vector if eng == "v" else nc.gpsimd
                    e.tensor_copy(t[:, k, :], st[:, o:o + m])
                    if extra is not None:
                        extra(k)
                return t

            # interleave nd-cast after each wcat-cast: f1's k-th MM needs both
            def _nd_cast(k):
                nc.vector.tensor_copy(nd_r[:, k, :], nd_f[:, k, :])

            wcat = load_w("wcat", 5, H, "v", extra=_nd_cast)
            wb = load_w("wb", 4, H, "v")
            w1 = load_w("w1", 4, H, "v")
            w2 = load_w("w2", 4, H, "v")
            wcb = load_w("wcb", 4, H, "v")
            wcc = load_w("wcc", 4, H, "v")
            wout = load_w("wout", 4, P, "v")

            # ---- node phase: f1, f2, then u = W1@f2 (all nodes) and
            #      vpb = W2@f2 + b_ca (per-partition bias source for e1) ----
            f1 = wr.tile([P, 4, NPAD], f32r, tag="f1")
            for m in range(4):
                pt = psA.tile([P, CHUNK], f32, tag="psA")
                for k in range(5):
                    nc.tensor.matmul(pt[:, :NPAD], wcat[:, k, m * P:(m + 1) * P],
                                     nd_r[:, k, :], start=(k == 0), stop=(k == 4))
                nc.scalar.activation(f1[:, m, :], pt[:, :NPAD], Relu,
                                     bias=bias("b1", m), scale=1.0)

            f2 = wr.tile([P, 4, NPAD], f32r, tag="f2")
            for m in range(4):
                pt = psB.tile([P, CHUNK], f32, tag="psB")
                for k in range(4):
                    nc.tensor.matmul(pt[:, :NPAD], wb[:, k, m * P:(m + 1) * P],
                                     f1[:, k, :], start=(k == 0), stop=(k == 3))
                nc.scalar.activation(f2[:, m, :], pt[:, :NPAD], Relu,
                                     bias=bias("bb", m), scale=1.0)

            u = wr.tile([P, 4, NPAD], f32, tag="u")
            vpb = wr.tile([P, 4, NPAD], f32, tag="vpb")
            # interleave u/v per k-tile so e1 of chunk 0 can start early
            for mm in range(4):
                pu = psA.tile([P, CHUNK], f32, tag="psA")
                for k in range(4):
                    nc.tensor.matmul(pu[:, :NPAD], w1[:, k, mm * P:(mm + 1) * P],
                                     f2[:, k, :], start=(k == 0), stop=(k == 3))
                nc.scalar.copy(u[:, mm, :], pu[:, :NPAD])
                pv = psB.tile([P, CHUNK], f32, tag="psB")
                for k in range(4):
                    nc.tensor.matmul(pv[:, :NPAD], w2[:, k, mm * P:(mm + 1) * P],
                                     f2[:, k, :], start=(k == 0), stop=(k == 3))
                nc.vector.tensor_scalar_add(vpb[:, mm, :], pv[:, :NPAD],
                                            bias("bca", mm))

            # ---- edge phase: 36 chunks x 512 flat edge columns ----
            for cc in range(NCHUNK):
                f0 = cc * CHUNK
                e1 = ep.tile([P, 4, CHUNK], f32r, tag="e1")
                r_lo = f0 // N
                r_hi = (f0 + CHUNK - 1) // N
                for kt in range(4):
                    for rl in range(r_lo, r_hi + 1):
                        cs = max(f0, rl * N)
                        ce = min(f0 + CHUNK, (rl + 1) * N)
                        nc.scalar.activation(
                            e1[:, kt, cs - f0:ce - f0],
                            u[:, kt, cs - rl * N:ce - rl * N],
                            Relu, bias=vpb[:, kt, rl:rl + 1], scale=1.0)

                e2 = ep.tile([P, 4, CHUNK], f32r, tag="e2")
                for m in range(4):
                    pt = psA.tile([P, CHUNK], f32, tag="psA")
                    for k in range(4):
                        nc.tensor.matmul(pt[:], wcb[:, k, m * P:(m + 1) * P],
                                         e1[:, k, :], start=(k == 0), stop=(k == 3))
                    nc.vector.tensor_scalar(e2[:, m, :], pt[:],
                                            bias("bcb", m), 0.0, add, amax)

                e3 = ep.tile([P, 4, CHUNK], f32r, tag="e3")
                for m in range(4):
                    pt = psB.tile([P, CHUNK], f32, tag="psB")
                    for k in range(4):
                        nc.tensor.matmul(pt[:], wcc[:, k, m * P:(m + 1) * P],
                                         e2[:, k, :], start=(k == 0), stop=(k == 3))
                    nc.vector.tensor_scalar(e3[:, m, :], pt[:],
                                            bias("bcc", m), 0.0, add, amax)

                po = psA.tile([P, CHUNK], f32, tag="psA")
                for k in range(4):
                    nc.tensor.matmul(po[:], wout[:, k, :], e3[:, k, :],
                                     start=(k == 0), stop=(k == 3))
                ob = outp.tile([2, CHUNK], f32, tag="ob")
                nc.vector.tensor_scalar_add(ob[:], po[:2, :], bias("bout", 0)[:2])
                nc.sync.dma_start(y[:, f0:f0 + CHUNK], ob[:])

    nc.compile()
    return nc


_cache = {}


def _get_nc():
    if "nc" not in _cache:
        _cache["nc"] = _build()
    return _cache["nc"]


def kernel(brick_vectors, xy, W_xy, b_xy, W_a, b_a, W_b, b_b,
           W_ca, b_ca, W_cb, b_cb, W_cc, b_cc, W_out, b_out):
    # force plain numpy up front (inputs may arrive as jax arrays)
    brick_vectors = np.asarray(brick_vectors, np.float32)
    xy = np.asarray(xy, np.float32)
    W_xy, b_xy, W_a, b_a = map(np.asarray, (W_xy, b_xy, W_a, b_a))
    W_b, b_b, W_ca, b_ca = map(np.asarray, (W_b, b_b, W_ca, b_ca))
    W_cb, b_cb, W_cc, b_cc = map(np.asarray, (W_cb, b_cb, W_cc, b_cc))
    W_out, b_out = np.asarray(W_out), np.asarray(b_out)
    blob, bblob = _pack_weights(W_xy, b_xy, W_a, b_a, W_b, b_b, W_ca, b_ca,
                                W_cb, b_cb, W_cc, b_cc, W_out, b_out)

    perms = []
    in_maps = []
    for c in range(NCORES):
        b, half = c // 2, c % 2
        perm = np.concatenate([np.arange(96) + 96 * half,
                               np.arange(96) + 96 * (1 - half)])
        perms.append((b, perm))
        in_maps.append({
            "wblob": blob,
            "bblob": bblob,
            "nodes": _pack_nodes(brick_vectors[b], xy[b], perm),
        })

    nc = _get_nc()
    res = run_bass_kernel_spmd(nc, in_maps, list(range(NCORES)))

    out = np.empty((B, N, N, 2), np.float32)
    for c in range(NCORES):
        b, perm = perms[c]
        yc = res.results[c]["y"].reshape(2, RLOC, N)       # [2, rl, jj]
        out[b][np.ix_(perm[:RLOC], perm)] = yc.transpose(1, 2, 0)
    return out


# revision 4
# speedup vs baseline: 1.0733x; 1.0054x over previous
"""Trainium2 Bass kernel for nn_BrickVectorEdgeModel (GNN edge MLP).

Computes, for each batch b and node pair (i, j):
    f   = relu(W_b @ relu(W_a @ bv + b_a + W_xy @ xy + b_xy) + b_b)   per node
    e1  = relu(W1 @ f[j] + W2 @ f[i] + b_ca)                          per edge
    e2  = relu(W_cb @ e1 + b_cb)
    e3  = relu(W_cc @ e2 + b_cc)
    out = W_out @ e3 + b_out                                          (2 channels)

Sharding: the (B=4, N=192) x N edge grid has 768 i-rows; each of the 8
cores takes 96 consecutive rows, which always fall inside a single batch
b = core//2.  Host permutes that batch's 192 nodes so the core's own 96
i-rows come first; every core then runs the identical program (SPMD) on
its own node set.  Matmuls run in fp16 (weights and activations; fp32
PSUM accumulate) -- full PE rate with FWL weight loads and ~1e-3-class
relative error, no on-chip rounding copies needed.
"""

import numpy as np

import concourse.bass as bass
import concourse.mybir as mybir
import concourse.tile as tile
from concourse import bacc
from concourse.bass_utils import run_bass_kernel_spmd

P = 128
H = 512          # hidden width
D = 512          # brick vector dim
B = 4
N = 192          # nodes per batch
NCORES = 8
RLOC = 96        # edge-grid rows per core
EDGES = RLOC * N             # flat edge columns per core (18432)
CHUNK = 512
NCHUNK = EDGES // CHUNK      # 36
NPAD = 256       # node-phase free dim (192 padded to 256 for full-rate f32r)

# weight blob layout (fp16): name -> (offset_cols, size_cols), [128 x WCOLS]
_layout = [
    ("wcat", 5 * H),   # [d_tile(4)+xy_pad(1), 512] stationary tiles for layer a
    ("wb", 4 * H),
    ("w1", 4 * H),
    ("w2", 4 * H),
    ("wcb", 4 * H),
    ("wcc", 4 * H),
    ("wout", 4 * P),   # W_out^T padded from [512,2] to [512,128]
]
OFF = {}
_c = 0
for _n, _s in _layout:
    OFF[_n] = (_c, _s)
    _c += _s
WCOLS = _c

# bias blob layout (fp32): [128 x BCOLS]
_blayout = [("b1", 4), ("bb", 4), ("bca", 4), ("bcb", 4), ("bcc", 4), ("bout", 1)]
BOFF = {}
_c = 0
for _n, _s in _blayout:
    BOFF[_n] = (_c, _s)
    _c += _s
BCOLS = _c


def _to_tiles(w):
    """[K, M] (K = 4*128 contraction) -> [128, 4, M] stationary layout."""
    K, M = w.shape
    return w.reshape(K // P, P, M).transpose(1, 0, 2)


def _pack_weights(W_xy, b_xy, W_a, b_a, W_b, b_b, W_ca, b_ca, W_cb, b_cb,
                  W_cc, b_cc, W_out, b_out):
    blob = np.zeros((P, WCOLS), np.float16)

    def put(name, arr3):  # arr3: [128, n_k, M]
        off, sz = OFF[name]
        blob[:, off:off + sz] = arr3.reshape(P, -1).astype(np.float16)

    wcat = np.zeros((P, 5, H), np.float32)
    wcat[:, :4, :] = _to_tiles(W_a.T.astype(np.float32))      # [512d, 512h]
    wcat[0:2, 4, :] = W_xy.T.astype(np.float32)               # [2, 512]
    put("wcat", wcat)
    put("wb", _to_tiles(W_b.T.astype(np.float32)))
    W1, W2 = W_ca[:, :H], W_ca[:, H:]
    put("w1", _to_tiles(W1.T.astype(np.float32)))
    put("w2", _to_tiles(W2.T.astype(np.float32)))
    put("wcb", _to_tiles(W_cb.T.astype(np.float32)))
    put("wcc", _to_tiles(W_cc.T.astype(np.float32)))
    wout = np.zeros((H, P), np.float32)
    wout[:, 0:2] = W_out.T.astype(np.float32)
    put("wout", _to_tiles(wout))

    bblob = np.zeros((P, BCOLS), np.float32)

    def putb(name, vec):  # [512] -> [128, 4]
        off, sz = BOFF[name]
        bblob[:, off:off + sz] = vec.astype(np.float32).reshape(4, P).T

    putb("b1", np.asarray(b_a) + np.asarray(b_xy))
    putb("bb", b_b)
    putb("bca", b_ca)
    putb("bcb", b_cb)
    putb("bcc", b_cc)
    off, _ = BOFF["bout"]
    bblob[0:2, off] = np.asarray(b_out, np.float32)
    return blob, bblob


def _pack_nodes(bv_b, xy_b, perm):
    """Per-core node blob [128, 5, NPAD] fp16: k-tiles 0-3 = bv^T, 4 = xy^T."""
    nb = np.zeros((P, 5, NPAD), np.float16)
    bvT = bv_b[perm].T.astype(np.float32)          # [512, 192]
    nb[:, 0:4, :N] = bvT.reshape(4, P, N).transpose(1, 0, 2).astype(np.float16)
    nb[0:2, 4, :N] = xy_b[perm].T.astype(np.float16)
    return nb


def _build():
    f32 = mybir.dt.float32
    f32r = mybir.dt.float32r
    Relu = mybir.ActivationFunctionType.Relu
    add = mybir.AluOpType.add
    amax = mybir.AluOpType.max

    f16 = mybir.dt.float16
    nc = bacc.Bacc(None, target_bir_lowering=False)
    wblob = nc.declare_dram_parameter("wblob", [P, WCOLS], f16, isOutput=False)
    bblob = nc.declare_dram_parameter("bblob", [P, BCOLS], f32, isOutput=False)
    nodes = nc.declare_dram_parameter("nodes", [P, 5, NPAD], f16, isOutput=False)
    y = nc.declare_dram_parameter("y", [2, EDGES], f32, isOutput=True)

    with tile.TileContext(nc) as tc:
        with tc.tile_pool(name="wf", bufs=1) as wf, \
             tc.tile_pool(name="stp", bufs=8) as stp, \
             tc.tile_pool(name="wr", bufs=1) as wr, \
             tc.tile_pool(name="ep", bufs=2) as ep, \
             tc.tile_pool(name="outp", bufs=3) as outp, \
             tc.tile_pool(name="psA", bufs=4, space="PSUM") as psA, \
             tc.tile_pool(name="psB", bufs=4, space="PSUM") as psB:

            # biases: small resident fp32 tile (declared below, loaded after
            # the critical-path weight stage A)
            bias_t = wf.tile([P, BCOLS], f32, tag="bias")

            def bias(name, m):
                off, _ = BOFF[name]
                return bias_t[:, off + m:off + m + 1]

            # nodes: fp16, used directly as matmul rhs (no rounding needed)
            nd_r = wf.tile([P, 5, NPAD], f16, tag="nodes")
            nc.sync.dma_start(nd_r[:], nodes[:])
            nc.sync.dma_start(bias_t[:], bblob[:])

            # weights: fp16 straight into lhsT tiles -- 3 batched DMAs
            # (~650ns trigger each on the Sync sequencer), no casts at all
            CUT1 = OFF["wb"][0]
            CUT2 = OFF["wcb"][0]
            stA = stp.tile([P, CUT1], f16, tag="stA")
            nc.sync.dma_start(stA[:], wblob[:, :CUT1])
            stB = stp.tile([P, CUT2 - CUT1], f16, tag="stB")
            nc.sync.dma_start(stB[:], wblob[:, CUT1:CUT2])
            stC = stp.tile([P, WCOLS - CUT2], f16, tag="stC")
            nc.sync.dma_start(stC[:], wblob[:, CUT2:])

            def wslice(name, nk, m):
                off, sz = OFF[name]
                assert sz == nk * m
                if off < CUT1:
                    st, base = stA, 0
                elif off < CUT2:
                    st, base = stB, CUT1
                else:
                    st, base = stC, CUT2
                return st[:, off - base:off - base + sz].rearrange(
                    "p (a b) -> p a b", b=m)

            wcat = wslice("wcat", 5, H)
            wb = wslice("wb", 4, H)
            w1 = wslice("w1", 4, H)
            w2 = wslice("w2", 4, H)
            wcb = wslice("wcb", 4, H)
            wcc = wslice("wcc", 4, H)
            wout = wslice("wout", 4, P)

            # ---- node phase: f1, f2, then u = W1@f2 (all nodes) and
            #      vpb = W2@f2 + b_ca (per-partition bias source for e1) ----
            f1 = wr.tile([P, 4, NPAD], f16, tag="f1")
            for m in range(4):
                pt = psA.tile([P, CHUNK], f32, tag="psA")
                for k in range(5):
                    nc.tensor.matmul(pt[:, :NPAD], wcat[:, k, m * P:(m + 1) * P],
                                     nd_r[:, k, :], start=(k == 0), stop=(k == 4))
                nc.scalar.activation(f1[:, m, :], pt[:, :NPAD], Relu,
                                     bias=bias("b1", m), scale=1.0)

            f2 = wr.tile([P, 4, NPAD], f16, tag="f2")
            for m in range(4):
                pt = psB.tile([P, CHUNK], f32, tag="psB")
                for k in range(4):
                    nc.tensor.matmul(pt[:, :NPAD], wb[:, k, m * P:(m + 1) * P],
                                     f1[:, k, :], start=(k == 0), stop=(k == 3))
                nc.scalar.activation(f2[:, m, :], pt[:, :NPAD], Relu,
                                     bias=bias("bb", m), scale=1.0)

            u = wr.tile([P, 4, NPAD], f32, tag="u")
            vpb = wr.tile([P, 4, NPAD], f32, tag="vpb")
            # interleave u/v per k-tile so e1 of chunk 0 can start early
            for mm in range(4):
                pu = psA.tile([P, CHUNK], f32, tag="psA")
                for k in range(4):
                    nc.tensor.matmul(pu[:, :NPAD], w1[:, k, mm * P:(mm + 1) * P],
                                     f2[:, k, :], start=(k == 0), stop=(k == 3))
                nc.scalar.copy(u[:, mm, :], pu[:, :NPAD])
                pv = psB.tile([P, CHUNK], f32, tag="psB")
                for k in range(4):
                    nc.tensor.matmul(pv[:, :NPAD], w2[:, k, mm * P:(mm + 1) * P],
                                     f2[:, k, :], start=(k == 0), stop=(k == 3))
                nc.vector.tensor_scalar_add(vpb[:, mm, :], pv[:, :NPAD],
                                            bias("bca", mm))

            # ---- edge phase: 36 chunks x 512 flat edge columns ----
            for cc in range(NCHUNK):
                f0 = cc * CHUNK
                e1 = ep.tile([P, 4, CHUNK], f16, tag="e1")
                r_lo = f0 // N
                r_hi = (f0 + CHUNK - 1) // N
                for kt in range(4):
                    for rl in range(r_lo, r_hi + 1):
                        cs = max(f0, rl * N)
                        ce = min(f0 + CHUNK, (rl + 1) * N)
                        nc.scalar.activation(
                            e1[:, kt, cs - f0:ce - f0],
                            u[:, kt, cs - rl * N:ce - rl * N],
                            Relu, bias=vpb[:, kt, rl:rl + 1], scale=1.0)

                e2 = ep.tile([P, 4, CHUNK], f16, tag="e2")
                for m in range(4):
                    pt = psA.tile([P, CHUNK], f32, tag="psA")
                    for k in range(4):
                        nc.tensor.matmul(pt[:], wcb[:, k, m * P:(m + 1) * P],
                                         e1[:, k, :], start=(k == 0), stop=(k == 3))
                    nc.vector.tensor_scalar(e2[:, m, :], pt[:],
                                            bias("bcb", m), 0.0, add, amax)

                e3 = ep.tile([P, 4, CHUNK], f16, tag="e3")
                for m in range(4):
                    pt = psB.tile([P, CHUNK], f32, tag="psB")
                    for k in range(4):
                        nc.tensor.matmul(pt[:], wcc[:, k, m * P:(m + 1) * P],
                                         e2[:, k, :], start=(k == 0), stop=(k == 3))
                    nc.vector.tensor_scalar(e3[:, m, :], pt[:],
                                            bias("bcc", m), 0.0, add, amax)

                po = psA.tile([P, CHUNK], f32, tag="psA")
                for k in range(4):
                    nc.tensor.matmul(po[:], wout[:, k, :], e3[:, k, :],
                                     start=(k == 0), stop=(k == 3))
                ob = outp.tile([2, CHUNK], f32, tag="ob")
                nc.vector.tensor_scalar_add(ob[:], po[:2, :], bias("bout", 0)[:2])
                nc.sync.dma_start(y[:, f0:f0 + CHUNK], ob[:])

    nc.compile()
    return nc


_cache = {}


def _get_nc():
    if "nc" not in _cache:
        _cache["nc"] = _build()
    return _cache["nc"]


def kernel(brick_vectors, xy, W_xy, b_xy, W_a, b_a, W_b, b_b,
           W_ca, b_ca, W_cb, b_cb, W_cc, b_cc, W_out, b_out):
    # force plain numpy up front (inputs may arrive as jax arrays)
    brick_vectors = np.asarray(brick_vectors, np.float32)
    xy = np.asarray(xy, np.float32)
    W_xy, b_xy, W_a, b_a = map(np.asarray, (W_xy, b_xy, W_a, b_a))
    W_b, b_b, W_ca, b_ca = map(np.asarray, (W_b, b_b, W_ca, b_ca))
    W_cb, b_cb, W_cc, b_cc = map(np.asarray, (W_cb, b_cb, W_cc, b_cc))
    W_out, b_out = np.asarray(W_out), np.asarray(b_out)
    blob, bblob = _pack_weights(W_xy, b_xy, W_a, b_a, W_b, b_b, W_ca, b_ca,
                                W_cb, b_cb, W_cc, b_cc, W_out, b_out)

    perms = []
    in_maps = []
    for c in range(NCORES):
        b, half = c // 2, c % 2
        perm = np.concatenate([np.arange(96) + 96 * half,
                               np.arange(96) + 96 * (1 - half)])
        perms.append((b, perm))
        in_maps.append({
            "wblob": blob,
            "bblob": bblob,
            "nodes": _pack_nodes(brick_vectors[b], xy[b], perm),
        })

    nc = _get_nc()
    res = run_bass_kernel_spmd(nc, in_maps, list(range(NCORES)))

    out = np.empty((B, N, N, 2), np.float32)
    for c in range(NCORES):
        b, perm = perms[c]
        yc = res.results[c]["y"].reshape(2, RLOC, N)       # [2, rl, jj]
        out[b][np.ix_(perm[:RLOC], perm)] = yc.transpose(1, 2, 0)
    return out
